# revision 25
# baseline (speedup 1.0000x reference)
"""GCN (3x GCNConv + BN + final linear) on 8 TRN2 NeuronCores.

Strategy:
- Pad N=50000 -> NP=50176 = 392 blocks of 128 nodes. Core c owns 49
  blocks (6272 nodes) and all edges whose destination (col) lies in them.
- The GCN norm dinv[row]*dinv[col] is factorized: dinv[row] is folded into
  edge_attr (host) and into the gather table rows (device); dinv[col] is
  applied to the aggregated block output (device).
- BatchNorm+bias are affine per-feature: the scale is folded into the next
  layer's weights on device (one 128x128 scalar op); the bias row is
  produced by a [1,128] matmul and applied as a rank-1 matmul into the
  node-linear PSUM group. The node linear is computed node-major
  (lhsT = t^T block stationary, rhs = W moving), so the shard is written
  node-major with no PE transposes.
- Edge phase per dest block: dma_gather of source rows (int16 indices,
  table split in two 25088-row halves), messages as dense matmuls
  (identity-add of gathered rows + ea@We accumulated in PSUM, relu on
  scalar), scatter via one-hot indicator matmuls accumulated in PSUM.
- Per layer: node linear -> AllGather bf16 table (Shared output) ->
  edge phase -> BN stats AllReduce -> fold affine into next weights.
"""

import sys

sys.path.insert(0, "/opt/trn_rl_repo")

import numpy as np
import ml_dtypes

import concourse.bass as bass
import concourse.tile as tile
from concourse import bacc, mybir
from concourse.bass_utils import run_bass_kernel_spmd

# ---------------- constants ----------------
NCORES = 8
D = 128
DE = 16
EPS = 1e-5
P = 128


def configure(n):
    """Set the node count; everything else derives from it."""
    global N, BLOCKS, NP, BPC, NSH, LOB, HIB, LOROWS, HIROWS, TLO, THI
    N = n
    BLOCKS = ((N + P - 1) // P + NCORES - 1) // NCORES * NCORES
    NP = BLOCKS * P
    BPC = BLOCKS // NCORES
    NSH = BPC * P
    # shard split for the two-step AllGather: lo = first LOB blocks of each
    # core's shard, hi = the rest.  Gather tables are the concatenations of
    # the per-core lo/hi pieces; both must stay under the int16 index range.
    LOB = BPC // 2
    HIB = BPC - LOB
    LOROWS = LOB * P
    HIROWS = HIB * P
    TLO = NCORES * LOROWS
    THI = NCORES * HIROWS
    assert TLO < 32768 and THI < 32768


configure(50000)
E = 1_600_000

dt = mybir.dt
AF = mybir.ActivationFunctionType
ALU = mybir.AluOpType

S_DTYPE = dt.bfloat16        # indicator matrix dtype


def _bf16(a):
    return np.asarray(a, dtype=np.float32).astype(ml_dtypes.bfloat16)


# ---------------- host-side edge preprocessing ----------------

def _preprocess(edge_index, edge_attr):
    """Sort/pad edges per (core, dest-block); build packed device arrays.

    Returns dict of per-core numpy arrays + the uniform chunk schedule.
    """
    row = np.asarray(edge_index[0], dtype=np.int64)
    col = np.asarray(edge_index[1], dtype=np.int64)
    deg = np.bincount(row, minlength=N).astype(np.float32) + 1.0
    dinv = deg ** -0.5                                  # [N]
    ea_s = np.asarray(edge_attr, np.float32) * dinv[row][:, None]   # [E,16]

    blk = col // P                                      # dest block of each edge
    order = np.argsort(blk, kind="stable")
    row_s, col_s, blk_s = row[order], col[order], blk[order]
    ea_sorted = ea_s[order]
    # boundaries per block
    starts = np.searchsorted(blk_s, np.arange(BLOCKS))
    ends = np.searchsorted(blk_s, np.arange(BLOCKS), side="right")

    # per (core, local block): split lo/hi rows, sizes
    lists = [[None] * BPC for _ in range(NCORES)]
    n_lo = np.zeros((NCORES, BPC), np.int64)
    n_hi = np.zeros((NCORES, BPC), np.int64)
    # relabel rows into the split-AllGather table layouts:
    #   lo table = concat_c shard_c[:LOROWS]  -> idx = c*LOROWS + u
    #   hi table = concat_c shard_c[LOROWS:]  -> idx = c*HIROWS + (u - LOROWS)
    csrc = row_s // NSH
    u = row_s - csrc * NSH
    lo_of = csrc * LOROWS + u
    hi_of = csrc * HIROWS + (u - LOROWS)
    for g in range(BLOCKS):
        c, b = divmod(g, BPC)
        s, e = starts[g], ends[g]
        lo_mask = u[s:e] < LOROWS
        lo_idx = np.nonzero(lo_mask)[0]
        hi_idx = np.nonzero(~lo_mask)[0]
        lists[c][b] = (s, lo_idx, hi_idx)
        n_lo[c, b] = len(lo_idx)
        n_hi[c, b] = len(hi_idx)

    # uniform chunk counts across cores
    m_lo = np.maximum(1, (n_lo.max(axis=0) + P - 1) // P).astype(int)   # [BPC]
    m_hi = np.maximum(1, (n_hi.max(axis=0) + P - 1) // P).astype(int)
    m_tot = m_lo + m_hi
    tot_chunks = int(m_tot.sum())
    chunk_off = np.zeros(BPC, int)
    chunk_off[1:] = np.cumsum(m_tot)[:-1]
    # idx columns (16-wrapped) offsets, in units of int16 columns
    s_lo = m_lo * 8
    s_hi = m_hi * 8
    s_tot = s_lo + s_hi
    tot_s = int(s_tot.sum())
    s_off = np.zeros(BPC, int)
    s_off[1:] = np.cumsum(s_tot)[:-1]

    # dma_gather call layout: per (block, half) calls of GMAX chunks; call
    # ci_lo[b][j] covers chunks [j*GMAX, ...) of the lo half, etc.  Per-core
    # valid-index counts let the ucode skip trailing padding (idx = -1).
    GMAX = 8
    ci_lo, ci_hi = [], []
    ci = 0
    for b in range(BPC):
        ci_lo.append(list(range(ci, ci + (m_lo[b] + GMAX - 1) // GMAX)))
        ci += len(ci_lo[-1])
        ci_hi.append(list(range(ci, ci + (m_hi[b] + GMAX - 1) // GMAX)))
        ci += len(ci_hi[-1])
    ncalls = ci

    per_core = []
    for c in range(NCORES):
        eaT = np.zeros((DE, tot_chunks, P), np.float32)
        colrel = np.full((P, tot_chunks), 255, np.uint8)
        idx16 = np.zeros((16, tot_s), np.int16)
        counts = np.zeros(ncalls, np.int32)
        for b in range(BPC):
            s, lo_idx, hi_idx = lists[c][b]
            co = chunk_off[b]
            for half, sub, m_half, half_chunk_base, relab in (
                (0, lo_idx, m_lo[b], 0, lo_of),
                (1, hi_idx, m_hi[b], m_lo[b], hi_of),
            ):
                g_sz = int(m_half) * P
                # trailing -1 padding is skipped by the ucode via
                # num_idxs_reg (the hr tile is memset beforehand)
                nreal = len(sub)
                rows_h = np.full(g_sz, -1, np.int64)
                rows_h[: nreal] = relab[s + sub]
                cis = ci_lo[b] if half == 0 else ci_hi[b]
                for j, cc in enumerate(cis):
                    w = min(GMAX, int(m_half) - j * GMAX) * P
                    cnt = max(0, min(w, nreal - j * GMAX * P))
                    if cnt == 0:
                        rows_h[j * GMAX * P] = 0
                        cnt = 1
                    counts[cc] = cnt
                cols_h = np.full(g_sz, 255, np.int64)    # pad col -> 255
                cols_h[: len(sub)] = col_s[s + sub] - (c * BPC + b) * P
                ea_h = np.zeros((g_sz, DE), np.float32)
                ea_h[: len(sub)] = ea_sorted[s + sub]
                ii = np.arange(g_sz)
                pp, jj = ii % P, ii // P
                eaT[:, co + half_chunk_base + jj, pp] = ea_h.T
                colrel[pp, co + half_chunk_base + jj] = cols_h
                # 16-wrapped idx at column offset
                so = s_off[b] + (0 if half == 0 else s_lo[b])
                idx16[ii % 16, so + ii // 16] = rows_h
        per_core.append(
            dict(
                eaT=_bf16(eaT),
                colrel=colrel,
                idx16=np.tile(idx16, (8, 1)),            # replicate to 128 partitions
                gcounts=counts.reshape(1, ncalls),
            )
        )

    sched = dict(
        m_lo=[int(v) for v in m_lo], m_hi=[int(v) for v in m_hi],
        chunk_off=[int(v) for v in chunk_off], s_off=[int(v) for v in s_off],
        s_lo=[int(v) for v in s_lo],
        tot_chunks=tot_chunks, tot_s=tot_s,
        ci_lo=ci_lo, ci_hi=ci_hi, ncalls=ncalls, gmax=GMAX,
    )
    return per_core, sched, dinv


# ---------------- device program ----------------

def _build(sched):
    import os
    nc = bacc.Bacc(None, target_bir_lowering=False, debug=False)
    TC, TS = sched["tot_chunks"], sched["tot_s"]
    GMAX = sched["gmax"]                       # chunks per dma_gather call
    NCALLS = sched["ncalls"]
    ci_lo, ci_hi = sched["ci_lo"], sched["ci_hi"]
    SINGLE_PKT = os.environ.get("KSINGLEPKT", "0") == "1"
    USE_CNT = os.environ.get("KCNT", "1") == "1"
    SKIP_GATHER = os.environ.get("KSKIP_GATHER") == "1"
    SKIP_CC = os.environ.get("KSKIP_CC") == "1"
    SKIP_EDGE = os.environ.get("KSKIP_EDGE") == "1"
    SKIP_EA = os.environ.get("KSKIP_EA") == "1"

    # ---- external inputs (per-core shapes) ----
    decl = nc.declare_dram_parameter
    xT = decl("xT", [P, NSH], dt.bfloat16, isOutput=False)
    eaT_d = decl("eaT", [DE, TC, P], dt.bfloat16, isOutput=False)
    colrel_d = decl("colrel", [P, TC], dt.uint8, isOutput=False)
    idx_d = decl("idx16", [P, TS], dt.int16, isOutput=False)
    dinv_d = decl("dinvt", [P, NSH], dt.bfloat16, isOutput=False)
    dinvnm_d = decl("dinv_nm", [P, BPC], dt.float32, isOutput=False)
    iota_d = decl("iota_u8", [P, P], dt.uint8, isOutput=False)
    ident_d = decl("ident", [P, P], dt.bfloat16, isOutput=False)
    W_d = [decl(f"W{k}", [D, D], dt.bfloat16, isOutput=False) for k in (1, 2, 3)]
    Wf_d = [decl(f"Wf{k}", [D, D], dt.float32, isOutput=False) for k in (2, 3)]
    Wl_d = decl("Wlin", [D, D], dt.bfloat16, isOutput=False)
    Wlf_d = decl("Wlinf", [D, D], dt.float32, isOutput=False)
    We_d = [decl(f"We{k}", [DE, D], dt.bfloat16, isOutput=False) for k in (1, 2, 3)]
    # b_tot[k] = b_k + be_k as row [1,128]; bf16 row for layer 1 direct use
    brow1bf_d = decl("brow1bf", [1, D], dt.bfloat16, isOutput=False)
    brow_d = [decl(f"brow{k}", [1, D], dt.float32, isOutput=False) for k in (2, 3)]
    blrow_d = decl("blrow", [1, D], dt.float32, isOutput=False)
    g_d = [decl(f"g{k}", [D, 1], dt.float32, isOutput=False) for k in (1, 2, 3)]
    bt_d = [decl(f"bt{k}", [D, 1], dt.float32, isOutput=False) for k in (1, 2, 3)]
    gcnt_d = decl("gcounts", [1, NCALLS], dt.int32, isOutput=False)
    out_d = decl("out_nm", [NSH, D], dt.float32, isOutput=True)

    rg = [list(range(NCORES))]

    with tile.TileContext(nc) as tc:
        import contextlib
        with contextlib.ExitStack() as ctx:
            ek = ctx.enter_context
            const = ek(tc.tile_pool(name="const", bufs=1))
            nodeb = ek(tc.tile_pool(name="nodeb", bufs=3))
            edge_ea = ek(tc.tile_pool(name="edge_ea", bufs=4))
            edge_idx = ek(tc.tile_pool(name="edge_idx", bufs=4))
            edge_hr = ek(tc.tile_pool(name="edge_hr", bufs=3))
            edge_msg = ek(tc.tile_pool(name="edge_msg", bufs=2))
            edge_S = ek(tc.tile_pool(name="edge_S", bufs=2))
            small = ek(tc.tile_pool(name="small", bufs=4))
            trp = ek(tc.tile_pool(name="trp", bufs=3))
            ps_mp = ek(tc.tile_pool(name="ps_mp", bufs=3, space="PSUM"))
            ps_conv = ek(tc.tile_pool(name="ps_conv", bufs=2, space="PSUM"))
            ps_node = ek(tc.tile_pool(name="ps_node", bufs=2, space="PSUM"))
            ps_row = ek(tc.tile_pool(name="ps_row", bufs=1, space="PSUM"))
            dram = ek(tc.tile_pool(name="dram", bufs=2, space="DRAM"))

            # ---- load constants ----
            def ld(pool, shape, dty, src, name):
                t = pool.tile(shape, dty, name=name)
                nc.sync.dma_start(out=t[:], in_=src[...])
                return t

            xT_t = ld(const, [P, NSH], dt.bfloat16, xT, 'xT_t')
            W_t = [ld(const, [D, D], dt.bfloat16, W_d[i], f'W_t{i}') for i in range(3)]
            brow1bf_t = ld(const, [1, D], dt.bfloat16, brow1bf_d, 'brow1bf_t')
            dinvnm_t = ld(const, [P, BPC], dt.float32, dinvnm_d, 'dinvnm_t')
            dinv_t = ld(const, [P, NSH], dt.bfloat16, dinv_d, 'dinv_d_t')
            iota_t = ld(const, [P, P], dt.uint8, iota_d, 'iota_d_t')
            ident_t = ld(const, [P, P], dt.bfloat16, ident_d, 'ident_d_t')
            colrel_t = ld(const, [P, TC], dt.uint8, colrel_d, 'colrel_d_t')
            Wf_t = [ld(const, [D, D], dt.float32, Wf_d[i], f'Wf_t{i}') for i in range(2)]
            Wl_t = ld(const, [D, D], dt.bfloat16, Wl_d, 'Wl_d_t')
            Wlf_t = ld(const, [D, D], dt.float32, Wlf_d, 'Wlf_d_t')
            We_t = [ld(const, [DE, D], dt.bfloat16, We_d[i], f'We_t{i}') for i in range(3)]
            brow_t = [ld(const, [1, D], dt.float32, brow_d[i], f'brow_t{i}') for i in range(2)]
            blrow_t = ld(const, [1, D], dt.float32, blrow_d, 'blrow_d_t')
            g_t = [ld(const, [D, 1], dt.float32, g_d[i], f'g_t{i}') for i in range(3)]
            bt_t = [ld(const, [D, 1], dt.float32, bt_d[i], f'bt_t{i}') for i in range(3)]

            t_T = [const.tile([P, NSH], dt.bfloat16, name=f't_T{i}') for i in range(2)]
            eps_t = const.tile([P, 1], dt.float32, name='eps_t')
            nc.vector.memset(eps_t[:], EPS)
            ones_t = const.tile([1, P], dt.bfloat16, name='ones_t')
            nc.vector.memset(ones_t[:], 1.0)
            gcnt_t = ld(const, [1, NCALLS], dt.int32, gcnt_d, 'gcnt_t')
            cnt_reg = nc.alloc_register(mybir.EngineType.Pool, "gcnt_reg")

            def gather_count(cc, default):
                if not USE_CNT:
                    return default
                nc.gpsimd.reg_load(cnt_reg, gcnt_t[0:1, cc:cc + 1])
                return cnt_reg

            m_lo, m_hi = sched["m_lo"], sched["m_hi"]
            chunk_off, s_off, s_lo = sched["chunk_off"], sched["s_off"], sched["s_lo"]

            def node_linear(rhs_w, brow_bf, src_t, shard, final=False,
                            after_block=None):
                """Per block: psum = src_blk.T @ W + ones x brow; scale by
                dinv (not for final); write node-major to shard/out."""
                for b in range(BPC):
                    pp = ps_node.tile([P, P], dt.float32, space="PSUM",
                                      padded_shape=[P, 512])
                    nc.tensor.matmul(out=pp[:], lhsT=src_t[:, b * P:(b + 1) * P],
                                     rhs=rhs_w[:], start=True, stop=False,
                                     skip_group_check=True)
                    nc.tensor.matmul(out=pp[:], lhsT=ones_t[:], rhs=brow_bf[:],
                                     start=False, stop=True, skip_group_check=True)
                    if final:
                        ot = nodeb.tile([P, P], dt.float32)
                        nc.scalar.activation(out=ot[:], in_=pp[:], func=AF.Identity)
                    else:
                        ot = nodeb.tile([P, P], dt.bfloat16)
                        nc.scalar.activation(out=ot[:], in_=pp[:], func=AF.Identity,
                                             scale=dinvnm_t[:, b:b + 1])
                    nc.sync.dma_start(out=shard[b * P:(b + 1) * P, :], in_=ot[:])
                    if after_block and b in after_block:
                        after_block[b]()

            for k in range(3):  # layers 1..3
                # ---- fold previous BN (k>=1) into this layer's weights ----
                if k == 0:
                    Wp_t, brow_bf = W_t[0], brow1bf_t
                    src_t = xT_t
                else:
                    a_t, c_t = bn_fold  # from previous layer epilogue
                    Wp_t = small.tile([D, D], dt.bfloat16)
                    nc.scalar.activation(out=Wp_t[:], in_=W_t[k][:], func=AF.Identity,
                                         scale=a_t[:])
                    pb = ps_row.tile([1, D], dt.float32, space="PSUM",
                                     padded_shape=[1, 512])
                    nc.tensor.matmul(out=pb[:], lhsT=c_t[:], rhs=Wf_t[k - 1][:],
                                     start=True, stop=True)
                    browf = small.tile([1, D], dt.float32)
                    nc.vector.tensor_tensor(out=browf[:], in0=pb[:],
                                            in1=brow_t[k - 1][:], op=ALU.add)
                    brow_bf = small.tile([1, D], dt.bfloat16)
                    nc.scalar.activation(out=brow_bf[:], in_=browf[:], func=AF.Identity)
                    src_t = t_T[(k - 1) % 2]

                # ---- node linear (node-major shard) + split AllGather ----
                shard = dram.tile([NSH, D], dt.bfloat16)
                table_lo = dram.tile([TLO, D], dt.bfloat16, addr_space="Shared")
                table_hi = dram.tile([THI, D], dt.bfloat16, addr_space="Shared")

                def ag(piece, tab):
                    if not SKIP_CC:
                        nc.gpsimd.collective_compute(
                            "AllGather", ALU.bypass, replica_groups=rg,
                            ins=[piece.opt()], outs=[tab[:].opt()],
                        )
                    else:
                        nc.sync.dma_start(out=tab[:piece.shape[0], :], in_=piece)

                node_linear(Wp_t, brow_bf, src_t, shard, after_block={
                    LOB - 1: lambda: ag(shard[:LOROWS, :], table_lo),
                    BPC - 1: lambda: ag(shard[LOROWS:, :], table_hi),
                })

                # ---- edge phase over 49 dest blocks (software-pipelined:
                # lo gathers run DEPTH blocks ahead so the hi-table AllGather
                # and per-block compute hide under the gather stream) ----
                sums_t = small.tile([P, BPC], dt.float32)
                sqs_t = small.tile([P, BPC], dt.float32)
                tnew = t_T[k % 2]
                DEPTH = 2
                inflight = {}

                def stage_load_lo(b):
                    m = m_lo[b] + m_hi[b]
                    co = chunk_off[b]
                    ea_t = edge_ea.tile([DE, m, P], dt.bfloat16)
                    if not SKIP_EA:
                        nc.sync.dma_start(out=ea_t[:], in_=eaT_d[:, co:co + m, :])
                    else:
                        nc.vector.memset(ea_t[:, :1, :], 0.0)
                    stot_b = s_lo[b] + m_hi[b] * 8
                    idx_t = edge_idx.tile([P, stot_b], dt.int16, name=f'idxb')
                    nc.sync.dma_start(out=idx_t[:], in_=idx_d[:, s_off[b]:s_off[b] + stot_b])
                    hr_t = edge_hr.tile([P, m, D], dt.bfloat16)
                    if USE_CNT:
                        # skipped padding slots must hold finite values for
                        # the identity-add matmul (0 x one-hot stays 0)
                        nc.vector.memset(hr_t[:], 0.0)
                    if not SKIP_GATHER:
                        for j, pc in enumerate(range(0, m_lo[b], GMAX)):
                            pw = min(GMAX, m_lo[b] - pc)
                            nc.gpsimd.dma_gather(
                                out_ap=hr_t[:, pc:pc + pw, :],
                                in_ap=table_lo[:, :],
                                idxs_ap=idx_t[:, pc * 8:(pc + pw) * 8],
                                num_idxs=pw * P,
                                num_idxs_reg=gather_count(ci_lo[b][j], pw * P),
                                elem_size=D,
                                single_packet=SINGLE_PKT,
                            )
                    else:
                        nc.vector.memset(hr_t[:, :1, :], 0.25)
                    inflight[b] = (ea_t, idx_t, hr_t)

                def stage_compute(b):
                    m = m_lo[b] + m_hi[b]
                    co = chunk_off[b]
                    ea_t, idx_t, hr_t = inflight.pop(b)
                    if not SKIP_GATHER:
                        for j, pc in enumerate(range(0, m_hi[b], GMAX)):
                            pw = min(GMAX, m_hi[b] - pc)
                            nc.gpsimd.dma_gather(
                                out_ap=hr_t[:, m_lo[b] + pc:m_lo[b] + pc + pw, :],
                                in_ap=table_hi[:, :],
                                idxs_ap=idx_t[:, s_lo[b] + pc * 8:s_lo[b] + (pc + pw) * 8],
                                num_idxs=pw * P,
                                num_idxs_reg=gather_count(ci_hi[b][j], pw * P),
                                elem_size=D,
                                single_packet=SINGLE_PKT,
                            )
                    # S indicator [P, m, P]
                    S_t = edge_S.tile([P, m, P], S_DTYPE)
                    iota_b = bass.AP(tensor=iota_t.tensor, offset=iota_t[:].offset,
                                     ap=[iota_t[:].ap[0], [0, m], iota_t[:].ap[1]])
                    cr = colrel_t[:, co:co + m]
                    cr_b = bass.AP(tensor=colrel_t.tensor, offset=cr.offset,
                                   ap=[cr.ap[0], cr.ap[1], [0, P]])
                    nc.vector.tensor_tensor(out=S_t[:], in0=iota_b, in1=cr_b,
                                            op=ALU.is_equal)
                    # messages
                    msg_t = edge_msg.tile([P, m, D], dt.bfloat16)
                    if SKIP_EDGE:
                        nc.vector.memset(msg_t[:, :1, :], 0.1)
                    j = 0 if not SKIP_EDGE else m
                    while j < m:
                        jw = min(4, m - j)
                        mp = ps_mp.tile([P, 4, D], dt.float32, space="PSUM")
                        # start=True zeroes the whole 2KB bank, so the
                        # full-tile identity-add must come first.
                        nc.tensor.matmul(
                            out=mp[:, :jw, :].rearrange("p j d -> p (j d)"),
                            lhsT=ident_t[:],
                            rhs=hr_t[:, j:j + jw, :].rearrange("p j d -> p (j d)"),
                            start=True, stop=False, skip_group_check=True)
                        for jj in range(jw):
                            nc.tensor.matmul(out=mp[:, jj, :],
                                             lhsT=ea_t[:, j + jj, :], rhs=We_t[k][:],
                                             start=False, stop=(jj == jw - 1),
                                             skip_group_check=True)
                        nc.scalar.activation(
                            out=msg_t[:, j:j + jw, :].rearrange("p j d -> p (j d)"),
                            in_=mp[:, :jw, :].rearrange("p j d -> p (j d)"), func=AF.Relu)
                        j += jw
                    # scatter into conv accumulator (feature-major out)
                    cp = ps_conv.tile([P, P], dt.float32, space="PSUM",
                                      padded_shape=[P, 512])
                    for j in range(m if not SKIP_EDGE else 1):
                        nc.tensor.matmul(out=cp[:], lhsT=msg_t[:, j, :],
                                         rhs=S_t[:, j, :],
                                         start=(j == 0), stop=(j == m - 1))
                    # epilogue: dinv scale, relu -> t, stats
                    sl = slice(b * P, (b + 1) * P)
                    pre = trp.tile([P, P], dt.float32)
                    nc.vector.tensor_tensor(out=pre[:], in0=cp[:],
                                            in1=dinv_t[:, sl], op=ALU.mult)
                    nc.scalar.activation(out=tnew[:, sl], in_=pre[:], func=AF.Relu,
                                         accum_out=sums_t[:, b:b + 1])
                    sq_scr = trp.tile([P, P], dt.bfloat16)
                    nc.scalar.activation(out=sq_scr[:], in_=tnew[:, sl], func=AF.Square,
                                         accum_out=sqs_t[:, b:b + 1])

                for b in range(BPC + DEPTH):
                    if b < BPC:
                        stage_load_lo(b)
                    if b >= DEPTH:
                        stage_compute(b - DEPTH)

                # ---- BN stats + fold coefficients ----
                st = small.tile([P, 2], dt.float32)
                nc.vector.tensor_reduce(out=st[:, 0:1], in_=sums_t[:],
                                        axis=mybir.AxisListType.X, op=ALU.add)
                nc.vector.tensor_reduce(out=st[:, 1:2], in_=sqs_t[:],
                                        axis=mybir.AxisListType.X, op=ALU.add)
                st_in = dram.tile([P, 2], dt.float32)
                st_out = dram.tile([P, 2], dt.float32)
                nc.sync.dma_start(out=st_in[:], in_=st[:])
                if not SKIP_CC:
                    nc.gpsimd.collective_compute(
                        "AllReduce", ALU.add, replica_groups=rg,
                        ins=[st_in[:].opt()], outs=[st_out[:].opt()],
                    )
                else:
                    nc.sync.dma_start(out=st_out[:, :], in_=st_in[:, :])
                stg = small.tile([P, 2], dt.float32)
                nc.sync.dma_start(out=stg[:], in_=st_out[:])
                mu = small.tile([P, 1], dt.float32)
                nc.vector.tensor_scalar(out=mu[:], in0=stg[:, 0:1], scalar1=1.0 / N,
                                        scalar2=None, op0=ALU.mult)
                ex2 = small.tile([P, 1], dt.float32)
                nc.vector.tensor_scalar(out=ex2[:], in0=stg[:, 1:2], scalar1=1.0 / N,
                                        scalar2=None, op0=ALU.mult)
                var = small.tile([P, 1], dt.float32)
                nc.vector.tensor_tensor(out=var[:], in0=mu[:], in1=mu[:], op=ALU.mult)
                nc.vector.tensor_tensor(out=var[:], in0=ex2[:], in1=var[:],
                                        op=ALU.subtract)
                sd = small.tile([P, 1], dt.float32)
                nc.scalar.activation(out=sd[:], in_=var[:], func=AF.Sqrt, bias=eps_t[:])
                rs = small.tile([P, 1], dt.float32)
                nc.vector.reciprocal(out=rs[:], in_=sd[:])
                a_t = small.tile([P, 1], dt.float32)
                nc.vector.tensor_tensor(out=a_t[:], in0=rs[:], in1=g_t[k][:],
                                        op=ALU.mult)
                c_t = small.tile([P, 1], dt.float32)
                nc.vector.tensor_tensor(out=c_t[:], in0=mu[:], in1=a_t[:], op=ALU.mult)
                nc.vector.tensor_tensor(out=c_t[:], in0=bt_t[k][:], in1=c_t[:],
                                        op=ALU.subtract)
                bn_fold = (a_t, c_t)

            # ---- final linear: out = t3_blk.T @ Wl' + ones x bl' (node-major) ----
            a_t, c_t = bn_fold
            Wlp = small.tile([D, D], dt.bfloat16)
            nc.scalar.activation(out=Wlp[:], in_=Wl_t[:], func=AF.Identity, scale=a_t[:])
            pb = ps_row.tile([1, D], dt.float32, space="PSUM", padded_shape=[1, 512])
            nc.tensor.matmul(out=pb[:], lhsT=c_t[:], rhs=Wlf_t[:], start=True, stop=True)
            blrowf = small.tile([1, D], dt.float32)
            nc.vector.tensor_tensor(out=blrowf[:], in0=pb[:], in1=blrow_t[:], op=ALU.add)
            blrow_bf = small.tile([1, D], dt.bfloat16)
            nc.scalar.activation(out=blrow_bf[:], in_=blrowf[:], func=AF.Identity)
            node_linear(Wlp, blrow_bf, t_T[0], out_d, final=True)

    nc.finalize()
    return nc


# ---------------- public entry point ----------------

_CACHE = {}
LAST_EXEC_NS = None


def _make_in_maps(inputs, per_core, dinv):
    x = np.asarray(inputs["x"], np.float32)

    dinv_pad = np.zeros(NP, np.float32)
    dinv_pad[:N] = dinv
    xT_full = np.zeros((P, NP), np.float32)
    xT_full[:, :N] = x.T

    Ws = {k: np.asarray(inputs[k], np.float32) for k in
          ("W1", "W2", "W3", "Wl", "We1", "We2", "We3")}
    bt_tot = {k: np.asarray(inputs[f"b{k}"], np.float32) +
                 np.asarray(inputs[f"be{k}"], np.float32) for k in (1, 2, 3)}

    in_maps = []
    for c in range(NCORES):
        sl = slice(c * NSH, (c + 1) * NSH)
        im = dict(per_core[c])
        im["xT"] = _bf16(xT_full[:, sl])
        im["dinvt"] = _bf16(np.tile(dinv_pad[sl][None, :], (P, 1)))
        im["dinv_nm"] = np.ascontiguousarray(
            dinv_pad[sl].reshape(BPC, P).T.astype(np.float32))
        im["iota_u8"] = np.tile(np.arange(P, dtype=np.uint8)[None, :], (P, 1))
        im["ident"] = _bf16(np.eye(P))
        for i, k in enumerate((1, 2, 3)):
            im[f"W{k}"] = _bf16(Ws[f"W{k}"])
            im[f"We{k}"] = _bf16(Ws[f"We{k}"])
            im[f"g{k}"] = np.asarray(inputs[f"g{k}"], np.float32).reshape(D, 1)
            im[f"bt{k}"] = np.asarray(inputs[f"bt{k}"], np.float32).reshape(D, 1)
        im["Wf2"] = Ws["W2"]
        im["Wf3"] = Ws["W3"]
        im["Wlin"] = _bf16(Ws["Wl"])
        im["Wlinf"] = Ws["Wl"]
        im["brow1bf"] = _bf16(bt_tot[1].reshape(1, D))
        im["brow2"] = bt_tot[2].reshape(1, D)
        im["brow3"] = bt_tot[3].reshape(1, D)
        im["blrow"] = np.asarray(inputs["bl"], np.float32).reshape(1, D)
        in_maps.append(im)
    return in_maps


def kernel(**inputs):
    edge_attr = np.asarray(inputs["edge_attr"], np.float32)
    edge_index = np.asarray(inputs["edge_index"])

    per_core, sched, dinv = _preprocess(edge_index, edge_attr)
    in_maps = _make_in_maps(inputs, per_core, dinv)

    import os
    key = ("k", os.environ.get("KCNT", "1"), os.environ.get("KSINGLEPKT", "1"), sched["tot_chunks"], sched["tot_s"],
           tuple(sched["m_lo"]), tuple(sched["m_hi"]))
    if key not in _CACHE:
        _CACHE[key] = _build(sched)
    nc = _CACHE[key]

    trace = os.environ.get("KPROF") == "1"
    tdir = os.environ.get("KTRACEDIR") or None
    r = run_bass_kernel_spmd(nc, in_maps, core_ids=list(range(NCORES)), trace=trace,
                             tmpdir=tdir)
    if trace:
        print(f"HW exec time: {r.exec_time_ns} ns", flush=True)
        global LAST_EXEC_NS
        LAST_EXEC_NS = r.exec_time_ns
    res = r.results
    out = np.concatenate([res[c]["out_nm"] for c in range(NCORES)], axis=0)  # [NP, D]
    return np.ascontiguousarray(out[:N]).astype(np.float32)


# revision 28
# speedup vs baseline: 1.2814x; 1.2814x over previous
"""GCN (3x GCNConv + BN + final linear) on 8 TRN2 NeuronCores.

Strategy:
- Pad N=50000 -> NP=50176 = 392 blocks of 128 nodes. Core c owns 49
  blocks (6272 nodes) and all edges whose destination (col) lies in them.
- The GCN norm dinv[row]*dinv[col] is factorized: dinv[row] is folded into
  edge_attr (host) and into the gather table rows (device); dinv[col] is
  applied to the aggregated block output (device).
- BatchNorm+bias are affine per-feature: the scale is folded into the next
  layer's weights on device (one 128x128 scalar op); the bias row is
  produced by a [1,128] matmul and applied as a rank-1 matmul into the
  node-linear PSUM group. The node linear is computed node-major
  (lhsT = t^T block stationary, rhs = W moving), so the shard is written
  node-major with no PE transposes.
- Edge phase per dest block: dma_gather of source rows (int16 indices,
  table split in two 25088-row halves), messages as dense matmuls
  (identity-add of gathered rows + ea@We accumulated in PSUM, relu on
  scalar), scatter via one-hot indicator matmuls accumulated in PSUM.
- Per layer: node linear -> AllGather bf16 table (Shared output) ->
  edge phase -> BN stats AllReduce -> fold affine into next weights.
"""

import sys

sys.path.insert(0, "/opt/trn_rl_repo")

import numpy as np
import ml_dtypes

import concourse.bass as bass
import concourse.tile as tile
from concourse import bacc, mybir
from concourse.bass_utils import run_bass_kernel_spmd

# ---------------- constants ----------------
NCORES = 8
D = 128
DE = 16
EPS = 1e-5
P = 128


def configure(n):
    """Set the node count; everything else derives from it."""
    global N, BLOCKS, NP, BPC, NSH, LOB, HIB, LOROWS, HIROWS, TLO, THI
    N = n
    BLOCKS = ((N + P - 1) // P + NCORES - 1) // NCORES * NCORES
    NP = BLOCKS * P
    BPC = BLOCKS // NCORES
    NSH = BPC * P
    # shard split for the two-step AllGather: lo = first LOB blocks of each
    # core's shard, hi = the rest.  Gather tables are the concatenations of
    # the per-core lo/hi pieces; both must stay under the int16 index range.
    LOB = BPC // 2
    HIB = BPC - LOB
    LOROWS = LOB * P
    HIROWS = HIB * P
    TLO = NCORES * LOROWS
    THI = NCORES * HIROWS
    assert TLO < 32768 and THI < 32768


configure(50000)
E = 1_600_000

dt = mybir.dt
AF = mybir.ActivationFunctionType
ALU = mybir.AluOpType

S_DTYPE = dt.bfloat16        # indicator matrix dtype


def _bf16(a):
    return np.asarray(a, dtype=np.float32).astype(ml_dtypes.bfloat16)


# ---------------- host-side edge preprocessing ----------------

def _preprocess(edge_index, edge_attr):
    """Sort/pad edges per (core, dest-block); build packed device arrays.

    Returns dict of per-core numpy arrays + the uniform chunk schedule.
    """
    import os
    PAD_SKIP = os.environ.get("KCNT", "0") == "1"
    row = np.asarray(edge_index[0], dtype=np.int64)
    col = np.asarray(edge_index[1], dtype=np.int64)
    deg = np.bincount(row, minlength=N).astype(np.float32) + 1.0
    dinv = deg ** -0.5                                  # [N]
    ea_s = np.asarray(edge_attr, np.float32) * dinv[row][:, None]   # [E,16]

    blk = col // P                                      # dest block of each edge
    order = np.argsort(blk, kind="stable")
    row_s, col_s, blk_s = row[order], col[order], blk[order]
    ea_sorted = ea_s[order]
    # boundaries per block
    starts = np.searchsorted(blk_s, np.arange(BLOCKS))
    ends = np.searchsorted(blk_s, np.arange(BLOCKS), side="right")

    # per (core, local block): split lo/hi rows, sizes
    lists = [[None] * BPC for _ in range(NCORES)]
    n_lo = np.zeros((NCORES, BPC), np.int64)
    n_hi = np.zeros((NCORES, BPC), np.int64)
    # relabel rows into the split-AllGather table layouts:
    #   lo table = concat_c shard_c[:LOROWS]  -> idx = c*LOROWS + u
    #   hi table = concat_c shard_c[LOROWS:]  -> idx = c*HIROWS + (u - LOROWS)
    csrc = row_s // NSH
    u = row_s - csrc * NSH
    lo_of = csrc * LOROWS + u
    hi_of = csrc * HIROWS + (u - LOROWS)
    for g in range(BLOCKS):
        c, b = divmod(g, BPC)
        s, e = starts[g], ends[g]
        lo_mask = u[s:e] < LOROWS
        lo_idx = np.nonzero(lo_mask)[0]
        hi_idx = np.nonzero(~lo_mask)[0]
        lists[c][b] = (s, lo_idx, hi_idx)
        n_lo[c, b] = len(lo_idx)
        n_hi[c, b] = len(hi_idx)

    # uniform chunk counts across cores
    m_lo = np.maximum(1, (n_lo.max(axis=0) + P - 1) // P).astype(int)   # [BPC]
    m_hi = np.maximum(1, (n_hi.max(axis=0) + P - 1) // P).astype(int)
    m_tot = m_lo + m_hi
    tot_chunks = int(m_tot.sum())
    chunk_off = np.zeros(BPC, int)
    chunk_off[1:] = np.cumsum(m_tot)[:-1]
    # idx columns (16-wrapped) offsets, in units of int16 columns
    s_lo = m_lo * 8
    s_hi = m_hi * 8
    s_tot = s_lo + s_hi
    tot_s = int(s_tot.sum())
    s_off = np.zeros(BPC, int)
    s_off[1:] = np.cumsum(s_tot)[:-1]

    # dma_gather call layout: per (block, half) calls of GMAX chunks; call
    # ci_lo[b][j] covers chunks [j*GMAX, ...) of the lo half, etc.  Per-core
    # valid-index counts let the ucode skip trailing padding (idx = -1).
    GMAX = 8
    ci_lo, ci_hi = [], []
    ci = 0
    for b in range(BPC):
        ci_lo.append(list(range(ci, ci + (m_lo[b] + GMAX - 1) // GMAX)))
        ci += len(ci_lo[-1])
        ci_hi.append(list(range(ci, ci + (m_hi[b] + GMAX - 1) // GMAX)))
        ci += len(ci_hi[-1])
    ncalls = ci

    per_core = []
    for c in range(NCORES):
        eaT = np.zeros((DE, tot_chunks, P), np.float32)
        colrel = np.full((P, tot_chunks), 255, np.uint8)
        idx16 = np.zeros((16, tot_s), np.int16)
        counts = np.zeros(ncalls, np.int32)
        for b in range(BPC):
            s, lo_idx, hi_idx = lists[c][b]
            co = chunk_off[b]
            for half, sub, m_half, half_chunk_base, relab in (
                (0, lo_idx, m_lo[b], 0, lo_of),
                (1, hi_idx, m_hi[b], m_lo[b], hi_of),
            ):
                g_sz = int(m_half) * P
                # trailing -1 padding is skipped by the ucode via
                # num_idxs_reg (KCNT=1 only; measured net-negative: the
                # ucode's register read costs ~1us/call).  Default pads
                # with row 0 and full immediate counts.
                nreal = len(sub)
                rows_h = np.full(g_sz, -1 if PAD_SKIP else 0, np.int64)
                rows_h[: nreal] = relab[s + sub]
                cis = ci_lo[b] if half == 0 else ci_hi[b]
                for j, cc in enumerate(cis):
                    w = min(GMAX, int(m_half) - j * GMAX) * P
                    cnt = max(0, min(w, nreal - j * GMAX * P))
                    if cnt == 0:
                        if PAD_SKIP:
                            rows_h[j * GMAX * P] = 0
                        cnt = 1
                    counts[cc] = cnt
                cols_h = np.full(g_sz, 255, np.int64)    # pad col -> 255
                cols_h[: len(sub)] = col_s[s + sub] - (c * BPC + b) * P
                ea_h = np.zeros((g_sz, DE), np.float32)
                ea_h[: len(sub)] = ea_sorted[s + sub]
                ii = np.arange(g_sz)
                pp, jj = ii % P, ii // P
                eaT[:, co + half_chunk_base + jj, pp] = ea_h.T
                colrel[pp, co + half_chunk_base + jj] = cols_h
                # 16-wrapped idx at column offset
                so = s_off[b] + (0 if half == 0 else s_lo[b])
                idx16[ii % 16, so + ii // 16] = rows_h
        per_core.append(
            dict(
                eaT=_bf16(eaT),
                colrel=colrel,
                idx16=np.tile(idx16, (8, 1)),            # replicate to 128 partitions
                gcounts=counts.reshape(1, ncalls),
            )
        )

    sched = dict(
        m_lo=[int(v) for v in m_lo], m_hi=[int(v) for v in m_hi],
        chunk_off=[int(v) for v in chunk_off], s_off=[int(v) for v in s_off],
        s_lo=[int(v) for v in s_lo],
        tot_chunks=tot_chunks, tot_s=tot_s,
        ci_lo=ci_lo, ci_hi=ci_hi, ncalls=ncalls, gmax=GMAX,
    )
    return per_core, sched, dinv


# ---------------- device program ----------------

def _build(sched):
    import os
    NQ = int(os.environ.get("KQ", "1"))
    nc = bacc.Bacc(None, target_bir_lowering=False, debug=False,
                   num_swdge_queues=NQ)
    TC, TS = sched["tot_chunks"], sched["tot_s"]
    GMAX = sched["gmax"]                       # chunks per dma_gather call
    NCALLS = sched["ncalls"]
    ci_lo, ci_hi = sched["ci_lo"], sched["ci_hi"]
    SINGLE_PKT = os.environ.get("KSINGLEPKT", "0") == "1"
    USE_CNT = os.environ.get("KCNT", "0") == "1"
    SKIP_GATHER = os.environ.get("KSKIP_GATHER") == "1"
    SKIP_CC = os.environ.get("KSKIP_CC") == "1"
    SKIP_EDGE = os.environ.get("KSKIP_EDGE") == "1"
    SKIP_EA = os.environ.get("KSKIP_EA") == "1"

    # ---- external inputs (per-core shapes) ----
    decl = nc.declare_dram_parameter
    xT = decl("xT", [P, NSH], dt.bfloat16, isOutput=False)
    eaT_d = decl("eaT", [DE, TC, P], dt.bfloat16, isOutput=False)
    colrel_d = decl("colrel", [P, TC], dt.uint8, isOutput=False)
    idx_d = decl("idx16", [P, TS], dt.int16, isOutput=False)
    dinv_d = decl("dinvt", [P, NSH], dt.bfloat16, isOutput=False)
    dinvnm_d = decl("dinv_nm", [P, BPC], dt.float32, isOutput=False)
    iota_d = decl("iota_u8", [P, P], dt.uint8, isOutput=False)
    ident_d = decl("ident", [P, P], dt.bfloat16, isOutput=False)
    W_d = [decl(f"W{k}", [D, D], dt.bfloat16, isOutput=False) for k in (1, 2, 3)]
    Wf_d = [decl(f"Wf{k}", [D, D], dt.float32, isOutput=False) for k in (2, 3)]
    Wl_d = decl("Wlin", [D, D], dt.bfloat16, isOutput=False)
    Wlf_d = decl("Wlinf", [D, D], dt.float32, isOutput=False)
    We_d = [decl(f"We{k}", [DE, D], dt.bfloat16, isOutput=False) for k in (1, 2, 3)]
    # b_tot[k] = b_k + be_k as row [1,128]; bf16 row for layer 1 direct use
    brow1bf_d = decl("brow1bf", [1, D], dt.bfloat16, isOutput=False)
    brow_d = [decl(f"brow{k}", [1, D], dt.float32, isOutput=False) for k in (2, 3)]
    blrow_d = decl("blrow", [1, D], dt.float32, isOutput=False)
    g_d = [decl(f"g{k}", [D, 1], dt.float32, isOutput=False) for k in (1, 2, 3)]
    bt_d = [decl(f"bt{k}", [D, 1], dt.float32, isOutput=False) for k in (1, 2, 3)]
    gcnt_d = decl("gcounts", [1, NCALLS], dt.int32, isOutput=False)
    out_d = decl("out_nm", [NSH, D], dt.float32, isOutput=True)

    rg = [list(range(NCORES))]

    with tile.TileContext(nc) as tc:
        import contextlib
        with contextlib.ExitStack() as ctx:
            ek = ctx.enter_context
            const = ek(tc.tile_pool(name="const", bufs=1))
            nodeb = ek(tc.tile_pool(name="nodeb", bufs=3))
            edge_ea = ek(tc.tile_pool(name="edge_ea", bufs=4))
            edge_idx = ek(tc.tile_pool(name="edge_idx", bufs=4))
            edge_hr = ek(tc.tile_pool(name="edge_hr", bufs=3))
            edge_msg = ek(tc.tile_pool(name="edge_msg", bufs=2))
            edge_S = ek(tc.tile_pool(name="edge_S", bufs=2))
            small = ek(tc.tile_pool(name="small", bufs=4))
            trp = ek(tc.tile_pool(name="trp", bufs=3))
            ps_mp = ek(tc.tile_pool(name="ps_mp", bufs=3, space="PSUM"))
            ps_conv = ek(tc.tile_pool(name="ps_conv", bufs=2, space="PSUM"))
            ps_node = ek(tc.tile_pool(name="ps_node", bufs=2, space="PSUM"))
            ps_row = ek(tc.tile_pool(name="ps_row", bufs=1, space="PSUM"))
            dram = ek(tc.tile_pool(name="dram", bufs=2, space="DRAM"))

            # ---- load constants ----
            def ld(pool, shape, dty, src, name):
                t = pool.tile(shape, dty, name=name)
                nc.sync.dma_start(out=t[:], in_=src[...])
                return t

            xT_t = ld(const, [P, NSH], dt.bfloat16, xT, 'xT_t')
            W_t = [ld(const, [D, D], dt.bfloat16, W_d[i], f'W_t{i}') for i in range(3)]
            brow1bf_t = ld(const, [1, D], dt.bfloat16, brow1bf_d, 'brow1bf_t')
            dinvnm_t = ld(const, [P, BPC], dt.float32, dinvnm_d, 'dinvnm_t')
            dinv_t = ld(const, [P, NSH], dt.bfloat16, dinv_d, 'dinv_d_t')
            iota_t = ld(const, [P, P], dt.uint8, iota_d, 'iota_d_t')
            ident_t = ld(const, [P, P], dt.bfloat16, ident_d, 'ident_d_t')
            colrel_t = ld(const, [P, TC], dt.uint8, colrel_d, 'colrel_d_t')
            Wf_t = [ld(const, [D, D], dt.float32, Wf_d[i], f'Wf_t{i}') for i in range(2)]
            Wl_t = ld(const, [D, D], dt.bfloat16, Wl_d, 'Wl_d_t')
            Wlf_t = ld(const, [D, D], dt.float32, Wlf_d, 'Wlf_d_t')
            We_t = [ld(const, [DE, D], dt.bfloat16, We_d[i], f'We_t{i}') for i in range(3)]
            brow_t = [ld(const, [1, D], dt.float32, brow_d[i], f'brow_t{i}') for i in range(2)]
            blrow_t = ld(const, [1, D], dt.float32, blrow_d, 'blrow_d_t')
            g_t = [ld(const, [D, 1], dt.float32, g_d[i], f'g_t{i}') for i in range(3)]
            bt_t = [ld(const, [D, 1], dt.float32, bt_d[i], f'bt_t{i}') for i in range(3)]

            t_T = [const.tile([P, NSH], dt.bfloat16, name=f't_T{i}') for i in range(2)]
            eps_t = const.tile([P, 1], dt.float32, name='eps_t')
            nc.vector.memset(eps_t[:], EPS)
            ones_t = const.tile([1, P], dt.bfloat16, name='ones_t')
            nc.vector.memset(ones_t[:], 1.0)
            gcnt_t = ld(const, [1, NCALLS], dt.int32, gcnt_d, 'gcnt_t')
            cnt_reg = nc.alloc_register(mybir.EngineType.Pool, "gcnt_reg")

            def gather_count(cc, default):
                if not USE_CNT:
                    return default
                nc.gpsimd.reg_load(cnt_reg, gcnt_t[0:1, cc:cc + 1])
                return cnt_reg

            m_lo, m_hi = sched["m_lo"], sched["m_hi"]
            chunk_off, s_off, s_lo = sched["chunk_off"], sched["s_off"], sched["s_lo"]

            def node_linear(rhs_w, brow_bf, src_t, shard, final=False,
                            after_block=None):
                """Per block: psum = src_blk.T @ W + ones x brow; scale by
                dinv (not for final); write node-major to shard/out."""
                for b in range(BPC):
                    pp = ps_node.tile([P, P], dt.float32, space="PSUM",
                                      padded_shape=[P, 512])
                    nc.tensor.matmul(out=pp[:], lhsT=src_t[:, b * P:(b + 1) * P],
                                     rhs=rhs_w[:], start=True, stop=False,
                                     skip_group_check=True)
                    nc.tensor.matmul(out=pp[:], lhsT=ones_t[:], rhs=brow_bf[:],
                                     start=False, stop=True, skip_group_check=True)
                    if final:
                        ot = nodeb.tile([P, P], dt.float32)
                        nc.scalar.activation(out=ot[:], in_=pp[:], func=AF.Identity)
                    else:
                        ot = nodeb.tile([P, P], dt.bfloat16)
                        nc.scalar.activation(out=ot[:], in_=pp[:], func=AF.Identity,
                                             scale=dinvnm_t[:, b:b + 1])
                    nc.sync.dma_start(out=shard[b * P:(b + 1) * P, :], in_=ot[:])
                    if after_block and b in after_block:
                        after_block[b]()

            for k in range(3):  # layers 1..3
                # ---- fold previous BN (k>=1) into this layer's weights ----
                if k == 0:
                    Wp_t, brow_bf = W_t[0], brow1bf_t
                    src_t = xT_t
                else:
                    a_t, c_t = bn_fold  # from previous layer epilogue
                    Wp_t = small.tile([D, D], dt.bfloat16)
                    nc.scalar.activation(out=Wp_t[:], in_=W_t[k][:], func=AF.Identity,
                                         scale=a_t[:])
                    pb = ps_row.tile([1, D], dt.float32, space="PSUM",
                                     padded_shape=[1, 512])
                    nc.tensor.matmul(out=pb[:], lhsT=c_t[:], rhs=Wf_t[k - 1][:],
                                     start=True, stop=True)
                    browf = small.tile([1, D], dt.float32)
                    nc.vector.tensor_tensor(out=browf[:], in0=pb[:],
                                            in1=brow_t[k - 1][:], op=ALU.add)
                    brow_bf = small.tile([1, D], dt.bfloat16)
                    nc.scalar.activation(out=brow_bf[:], in_=browf[:], func=AF.Identity)
                    src_t = t_T[(k - 1) % 2]

                # ---- node linear (node-major shard) + split AllGather ----
                shard = dram.tile([NSH, D], dt.bfloat16)
                table_lo = dram.tile([TLO, D], dt.bfloat16, addr_space="Shared")
                table_hi = dram.tile([THI, D], dt.bfloat16, addr_space="Shared")

                def ag(piece, tab):
                    if not SKIP_CC:
                        nc.gpsimd.collective_compute(
                            "AllGather", ALU.bypass, replica_groups=rg,
                            ins=[piece.opt()], outs=[tab[:].opt()],
                        )
                    else:
                        nc.sync.dma_start(out=tab[:piece.shape[0], :], in_=piece)

                node_linear(Wp_t, brow_bf, src_t, shard, after_block={
                    LOB - 1: lambda: ag(shard[:LOROWS, :], table_lo),
                    BPC - 1: lambda: ag(shard[LOROWS:, :], table_hi),
                })

                # ---- edge phase over 49 dest blocks (software-pipelined:
                # lo gathers run DEPTH blocks ahead so the hi-table AllGather
                # and per-block compute hide under the gather stream) ----
                sums_t = small.tile([P, BPC], dt.float32)
                sqs_t = small.tile([P, BPC], dt.float32)
                tnew = t_T[k % 2]
                DEPTH = 2
                inflight = {}

                def stage_load_lo(b):
                    m = m_lo[b] + m_hi[b]
                    co = chunk_off[b]
                    ea_t = edge_ea.tile([DE, m, P], dt.bfloat16)
                    if not SKIP_EA:
                        nc.sync.dma_start(out=ea_t[:], in_=eaT_d[:, co:co + m, :])
                    else:
                        nc.vector.memset(ea_t[:, :1, :], 0.0)
                    stot_b = s_lo[b] + m_hi[b] * 8
                    idx_t = edge_idx.tile([P, stot_b], dt.int16, name=f'idxb')
                    nc.sync.dma_start(out=idx_t[:], in_=idx_d[:, s_off[b]:s_off[b] + stot_b])
                    hr_t = edge_hr.tile([P, m, D], dt.bfloat16)
                    if USE_CNT:
                        # skipped padding slots must hold finite values for
                        # the identity-add matmul (0 x one-hot stays 0)
                        nc.vector.memset(hr_t[:], 0.0)
                    if not SKIP_GATHER:
                        for j, pc in enumerate(range(0, m_lo[b], GMAX)):
                            pw = min(GMAX, m_lo[b] - pc)
                            nc.gpsimd.dma_gather(
                                out_ap=hr_t[:, pc:pc + pw, :],
                                in_ap=table_lo[:, :],
                                idxs_ap=idx_t[:, pc * 8:(pc + pw) * 8],
                                num_idxs=pw * P,
                                num_idxs_reg=gather_count(ci_lo[b][j], pw * P),
                                elem_size=D,
                                single_packet=SINGLE_PKT,
                                queue_num=ci_lo[b][j] % NQ,
                            )
                    else:
                        nc.vector.memset(hr_t[:, :1, :], 0.25)
                    inflight[b] = (ea_t, idx_t, hr_t)

                def stage_compute(b):
                    m = m_lo[b] + m_hi[b]
                    co = chunk_off[b]
                    ea_t, idx_t, hr_t = inflight.pop(b)
                    if not SKIP_GATHER:
                        for j, pc in enumerate(range(0, m_hi[b], GMAX)):
                            pw = min(GMAX, m_hi[b] - pc)
                            nc.gpsimd.dma_gather(
                                out_ap=hr_t[:, m_lo[b] + pc:m_lo[b] + pc + pw, :],
                                in_ap=table_hi[:, :],
                                idxs_ap=idx_t[:, s_lo[b] + pc * 8:s_lo[b] + (pc + pw) * 8],
                                num_idxs=pw * P,
                                num_idxs_reg=gather_count(ci_hi[b][j], pw * P),
                                elem_size=D,
                                single_packet=SINGLE_PKT,
                                queue_num=ci_hi[b][j] % NQ,
                            )
                    # S indicator [P, m, P]
                    S_t = edge_S.tile([P, m, P], S_DTYPE)
                    iota_b = bass.AP(tensor=iota_t.tensor, offset=iota_t[:].offset,
                                     ap=[iota_t[:].ap[0], [0, m], iota_t[:].ap[1]])
                    cr = colrel_t[:, co:co + m]
                    cr_b = bass.AP(tensor=colrel_t.tensor, offset=cr.offset,
                                   ap=[cr.ap[0], cr.ap[1], [0, P]])
                    nc.vector.tensor_tensor(out=S_t[:], in0=iota_b, in1=cr_b,
                                            op=ALU.is_equal)
                    # messages
                    msg_t = edge_msg.tile([P, m, D], dt.bfloat16)
                    if SKIP_EDGE:
                        nc.vector.memset(msg_t[:, :1, :], 0.1)
                    j = 0 if not SKIP_EDGE else m
                    while j < m:
                        jw = min(4, m - j)
                        mp = ps_mp.tile([P, 4, D], dt.float32, space="PSUM")
                        # start=True zeroes the whole 2KB bank, so the
                        # full-tile identity-add must come first.
                        nc.tensor.matmul(
                            out=mp[:, :jw, :].rearrange("p j d -> p (j d)"),
                            lhsT=ident_t[:],
                            rhs=hr_t[:, j:j + jw, :].rearrange("p j d -> p (j d)"),
                            start=True, stop=False, skip_group_check=True)
                        for jj in range(jw):
                            nc.tensor.matmul(out=mp[:, jj, :],
                                             lhsT=ea_t[:, j + jj, :], rhs=We_t[k][:],
                                             start=False, stop=(jj == jw - 1),
                                             skip_group_check=True)
                        nc.scalar.activation(
                            out=msg_t[:, j:j + jw, :].rearrange("p j d -> p (j d)"),
                            in_=mp[:, :jw, :].rearrange("p j d -> p (j d)"), func=AF.Relu)
                        j += jw
                    # scatter into conv accumulator (feature-major out)
                    cp = ps_conv.tile([P, P], dt.float32, space="PSUM",
                                      padded_shape=[P, 512])
                    for j in range(m if not SKIP_EDGE else 1):
                        nc.tensor.matmul(out=cp[:], lhsT=msg_t[:, j, :],
                                         rhs=S_t[:, j, :],
                                         start=(j == 0), stop=(j == m - 1))
                    # epilogue: dinv scale, relu -> t, stats
                    sl = slice(b * P, (b + 1) * P)
                    pre = trp.tile([P, P], dt.float32)
                    nc.vector.tensor_tensor(out=pre[:], in0=cp[:],
                                            in1=dinv_t[:, sl], op=ALU.mult)
                    nc.scalar.activation(out=tnew[:, sl], in_=pre[:], func=AF.Relu,
                                         accum_out=sums_t[:, b:b + 1])
                    sq_scr = trp.tile([P, P], dt.bfloat16)
                    nc.scalar.activation(out=sq_scr[:], in_=tnew[:, sl], func=AF.Square,
                                         accum_out=sqs_t[:, b:b + 1])

                for b in range(BPC + DEPTH):
                    if b < BPC:
                        stage_load_lo(b)
                    if b >= DEPTH:
                        stage_compute(b - DEPTH)

                # ---- BN stats + fold coefficients ----
                st = small.tile([P, 2], dt.float32)
                nc.vector.tensor_reduce(out=st[:, 0:1], in_=sums_t[:],
                                        axis=mybir.AxisListType.X, op=ALU.add)
                nc.vector.tensor_reduce(out=st[:, 1:2], in_=sqs_t[:],
                                        axis=mybir.AxisListType.X, op=ALU.add)
                st_in = dram.tile([P, 2], dt.float32)
                st_out = dram.tile([P, 2], dt.float32)
                nc.sync.dma_start(out=st_in[:], in_=st[:])
                if not SKIP_CC:
                    nc.gpsimd.collective_compute(
                        "AllReduce", ALU.add, replica_groups=rg,
                        ins=[st_in[:].opt()], outs=[st_out[:].opt()],
                    )
                else:
                    nc.sync.dma_start(out=st_out[:, :], in_=st_in[:, :])
                stg = small.tile([P, 2], dt.float32)
                nc.sync.dma_start(out=stg[:], in_=st_out[:])
                mu = small.tile([P, 1], dt.float32)
                nc.vector.tensor_scalar(out=mu[:], in0=stg[:, 0:1], scalar1=1.0 / N,
                                        scalar2=None, op0=ALU.mult)
                ex2 = small.tile([P, 1], dt.float32)
                nc.vector.tensor_scalar(out=ex2[:], in0=stg[:, 1:2], scalar1=1.0 / N,
                                        scalar2=None, op0=ALU.mult)
                var = small.tile([P, 1], dt.float32)
                nc.vector.tensor_tensor(out=var[:], in0=mu[:], in1=mu[:], op=ALU.mult)
                nc.vector.tensor_tensor(out=var[:], in0=ex2[:], in1=var[:],
                                        op=ALU.subtract)
                sd = small.tile([P, 1], dt.float32)
                nc.scalar.activation(out=sd[:], in_=var[:], func=AF.Sqrt, bias=eps_t[:])
                rs = small.tile([P, 1], dt.float32)
                nc.vector.reciprocal(out=rs[:], in_=sd[:])
                a_t = small.tile([P, 1], dt.float32)
                nc.vector.tensor_tensor(out=a_t[:], in0=rs[:], in1=g_t[k][:],
                                        op=ALU.mult)
                c_t = small.tile([P, 1], dt.float32)
                nc.vector.tensor_tensor(out=c_t[:], in0=mu[:], in1=a_t[:], op=ALU.mult)
                nc.vector.tensor_tensor(out=c_t[:], in0=bt_t[k][:], in1=c_t[:],
                                        op=ALU.subtract)
                bn_fold = (a_t, c_t)

            # ---- final linear: out = t3_blk.T @ Wl' + ones x bl' (node-major) ----
            a_t, c_t = bn_fold
            Wlp = small.tile([D, D], dt.bfloat16)
            nc.scalar.activation(out=Wlp[:], in_=Wl_t[:], func=AF.Identity, scale=a_t[:])
            pb = ps_row.tile([1, D], dt.float32, space="PSUM", padded_shape=[1, 512])
            nc.tensor.matmul(out=pb[:], lhsT=c_t[:], rhs=Wlf_t[:], start=True, stop=True)
            blrowf = small.tile([1, D], dt.float32)
            nc.vector.tensor_tensor(out=blrowf[:], in0=pb[:], in1=blrow_t[:], op=ALU.add)
            blrow_bf = small.tile([1, D], dt.bfloat16)
            nc.scalar.activation(out=blrow_bf[:], in_=blrowf[:], func=AF.Identity)
            node_linear(Wlp, blrow_bf, t_T[0], out_d, final=True)

    nc.finalize()
    return nc


# ---------------- public entry point ----------------

_CACHE = {}
LAST_EXEC_NS = None


def _make_in_maps(inputs, per_core, dinv):
    x = np.asarray(inputs["x"], np.float32)

    dinv_pad = np.zeros(NP, np.float32)
    dinv_pad[:N] = dinv
    xT_full = np.zeros((P, NP), np.float32)
    xT_full[:, :N] = x.T

    Ws = {k: np.asarray(inputs[k], np.float32) for k in
          ("W1", "W2", "W3", "Wl", "We1", "We2", "We3")}
    bt_tot = {k: np.asarray(inputs[f"b{k}"], np.float32) +
                 np.asarray(inputs[f"be{k}"], np.float32) for k in (1, 2, 3)}

    in_maps = []
    for c in range(NCORES):
        sl = slice(c * NSH, (c + 1) * NSH)
        im = dict(per_core[c])
        im["xT"] = _bf16(xT_full[:, sl])
        im["dinvt"] = _bf16(np.tile(dinv_pad[sl][None, :], (P, 1)))
        im["dinv_nm"] = np.ascontiguousarray(
            dinv_pad[sl].reshape(BPC, P).T.astype(np.float32))
        im["iota_u8"] = np.tile(np.arange(P, dtype=np.uint8)[None, :], (P, 1))
        im["ident"] = _bf16(np.eye(P))
        for i, k in enumerate((1, 2, 3)):
            im[f"W{k}"] = _bf16(Ws[f"W{k}"])
            im[f"We{k}"] = _bf16(Ws[f"We{k}"])
            im[f"g{k}"] = np.asarray(inputs[f"g{k}"], np.float32).reshape(D, 1)
            im[f"bt{k}"] = np.asarray(inputs[f"bt{k}"], np.float32).reshape(D, 1)
        im["Wf2"] = Ws["W2"]
        im["Wf3"] = Ws["W3"]
        im["Wlin"] = _bf16(Ws["Wl"])
        im["Wlinf"] = Ws["Wl"]
        im["brow1bf"] = _bf16(bt_tot[1].reshape(1, D))
        im["brow2"] = bt_tot[2].reshape(1, D)
        im["brow3"] = bt_tot[3].reshape(1, D)
        im["blrow"] = np.asarray(inputs["bl"], np.float32).reshape(1, D)
        in_maps.append(im)
    return in_maps


def kernel(**inputs):
    edge_attr = np.asarray(inputs["edge_attr"], np.float32)
    edge_index = np.asarray(inputs["edge_index"])

    per_core, sched, dinv = _preprocess(edge_index, edge_attr)
    in_maps = _make_in_maps(inputs, per_core, dinv)

    import os
    key = ("k", os.environ.get("KCNT", "0"), os.environ.get("KQ", "1"), os.environ.get("KSINGLEPKT", "0"), sched["tot_chunks"], sched["tot_s"],
           tuple(sched["m_lo"]), tuple(sched["m_hi"]))
    if key not in _CACHE:
        _CACHE[key] = _build(sched)
    nc = _CACHE[key]

    trace = os.environ.get("KPROF") == "1"
    tdir = os.environ.get("KTRACEDIR") or None
    r = run_bass_kernel_spmd(nc, in_maps, core_ids=list(range(NCORES)), trace=trace,
                             tmpdir=tdir)
    if trace:
        print(f"HW exec time: {r.exec_time_ns} ns", flush=True)
        global LAST_EXEC_NS
        LAST_EXEC_NS = r.exec_time_ns
    res = r.results
    out = np.concatenate([res[c]["out_nm"] for c in range(NCORES)], axis=0)  # [NP, D]
    return np.ascontiguousarray(out[:N]).astype(np.float32)


# revision 29
# speedup vs baseline: 1.2881x; 1.0052x over previous
"""GCN (3x GCNConv + BN + final linear) on 8 TRN2 NeuronCores.

Strategy:
- Pad N=50000 -> NP=50176 = 392 blocks of 128 nodes. Core c owns 49
  blocks (6272 nodes) and all edges whose destination (col) lies in them.
- The GCN norm dinv[row]*dinv[col] is factorized: dinv[row] is folded into
  edge_attr (host) and into the gather table rows (device); dinv[col] is
  applied to the aggregated block output (device).
- BatchNorm+bias are affine per-feature: the scale is folded into the next
  layer's weights on device (one 128x128 scalar op); the bias row is
  produced by a [1,128] matmul and applied as a rank-1 matmul into the
  node-linear PSUM group. The node linear is computed node-major
  (lhsT = t^T block stationary, rhs = W moving), so the shard is written
  node-major with no PE transposes.
- Edge phase per dest block: dma_gather of source rows (int16 indices;
  the table is split into lo/hi pieces under the int16 range), messages
  as dense matmuls (identity-add of gathered rows + ea@We accumulated in
  PSUM, relu on scalar), scatter via one-hot indicator matmuls
  accumulated in PSUM.
- The AllGather is split in two (lo/hi shard halves with a permuted
  table layout) so the hi AllGather and the per-block compute hide under
  the gather stream, which runs DEPTH blocks ahead (the gpsimd
  descriptor-generation for dma_gather is the kernel's bottleneck at
  ~7.5 ns/index; everything else is overlapped beneath it).
- Per layer: node linear -> 2x AllGather bf16 table (Shared output) ->
  edge phase -> BN stats AllReduce -> fold affine into next weights.
"""

import sys

sys.path.insert(0, "/opt/trn_rl_repo")

import numpy as np
import ml_dtypes

import concourse.bass as bass
import concourse.tile as tile
from concourse import bacc, mybir
from concourse.bass_utils import run_bass_kernel_spmd

# ---------------- constants ----------------
NCORES = 8
D = 128
DE = 16
EPS = 1e-5
P = 128


def configure(n):
    """Set the node count; everything else derives from it."""
    global N, BLOCKS, NP, BPC, NSH, LOB, HIB, LOROWS, HIROWS, TLO, THI
    N = n
    BLOCKS = ((N + P - 1) // P + NCORES - 1) // NCORES * NCORES
    NP = BLOCKS * P
    BPC = BLOCKS // NCORES
    NSH = BPC * P
    # shard split for the two-step AllGather: lo = first LOB blocks of each
    # core's shard, hi = the rest.  Gather tables are the concatenations of
    # the per-core lo/hi pieces; both must stay under the int16 index range.
    LOB = BPC // 2
    HIB = BPC - LOB
    LOROWS = LOB * P
    HIROWS = HIB * P
    TLO = NCORES * LOROWS
    THI = NCORES * HIROWS
    assert TLO < 32768 and THI < 32768


configure(50000)
E = 1_600_000

dt = mybir.dt
AF = mybir.ActivationFunctionType
ALU = mybir.AluOpType

S_DTYPE = dt.bfloat16        # indicator matrix dtype


def _bf16(a):
    return np.asarray(a, dtype=np.float32).astype(ml_dtypes.bfloat16)


# ---------------- host-side edge preprocessing ----------------

def _preprocess(edge_index, edge_attr):
    """Sort/pad edges per (core, dest-block); build packed device arrays.

    Returns dict of per-core numpy arrays + the uniform chunk schedule.
    """
    import os
    PAD_SKIP = os.environ.get("KCNT", "0") == "1"
    row = np.asarray(edge_index[0], dtype=np.int64)
    col = np.asarray(edge_index[1], dtype=np.int64)
    deg = np.bincount(row, minlength=N).astype(np.float32) + 1.0
    dinv = deg ** -0.5                                  # [N]
    ea_s = np.asarray(edge_attr, np.float32) * dinv[row][:, None]   # [E,16]

    blk = col // P                                      # dest block of each edge
    order = np.argsort(blk, kind="stable")
    row_s, col_s, blk_s = row[order], col[order], blk[order]
    ea_sorted = ea_s[order]
    # boundaries per block
    starts = np.searchsorted(blk_s, np.arange(BLOCKS))
    ends = np.searchsorted(blk_s, np.arange(BLOCKS), side="right")

    # per (core, local block): split lo/hi rows, sizes
    lists = [[None] * BPC for _ in range(NCORES)]
    n_lo = np.zeros((NCORES, BPC), np.int64)
    n_hi = np.zeros((NCORES, BPC), np.int64)
    # relabel rows into the split-AllGather table layouts:
    #   lo table = concat_c shard_c[:LOROWS]  -> idx = c*LOROWS + u
    #   hi table = concat_c shard_c[LOROWS:]  -> idx = c*HIROWS + (u - LOROWS)
    csrc = row_s // NSH
    u = row_s - csrc * NSH
    lo_of = csrc * LOROWS + u
    hi_of = csrc * HIROWS + (u - LOROWS)
    for g in range(BLOCKS):
        c, b = divmod(g, BPC)
        s, e = starts[g], ends[g]
        lo_mask = u[s:e] < LOROWS
        lo_idx = np.nonzero(lo_mask)[0]
        hi_idx = np.nonzero(~lo_mask)[0]
        lists[c][b] = (s, lo_idx, hi_idx)
        n_lo[c, b] = len(lo_idx)
        n_hi[c, b] = len(hi_idx)

    # uniform chunk counts across cores
    m_lo = np.maximum(1, (n_lo.max(axis=0) + P - 1) // P).astype(int)   # [BPC]
    m_hi = np.maximum(1, (n_hi.max(axis=0) + P - 1) // P).astype(int)
    m_tot = m_lo + m_hi
    tot_chunks = int(m_tot.sum())
    chunk_off = np.zeros(BPC, int)
    chunk_off[1:] = np.cumsum(m_tot)[:-1]
    # idx columns (16-wrapped) offsets, in units of int16 columns
    s_lo = m_lo * 8
    s_hi = m_hi * 8
    s_tot = s_lo + s_hi
    tot_s = int(s_tot.sum())
    s_off = np.zeros(BPC, int)
    s_off[1:] = np.cumsum(s_tot)[:-1]

    # dma_gather call layout: per (block, half) calls of GMAX chunks; call
    # ci_lo[b][j] covers chunks [j*GMAX, ...) of the lo half, etc.  Per-core
    # valid-index counts let the ucode skip trailing padding (idx = -1).
    GMAX = 8
    ci_lo, ci_hi = [], []
    ci = 0
    for b in range(BPC):
        ci_lo.append(list(range(ci, ci + (m_lo[b] + GMAX - 1) // GMAX)))
        ci += len(ci_lo[-1])
        ci_hi.append(list(range(ci, ci + (m_hi[b] + GMAX - 1) // GMAX)))
        ci += len(ci_hi[-1])
    ncalls = ci

    per_core = []
    for c in range(NCORES):
        eaT = np.zeros((DE, tot_chunks, P), np.float32)
        colrel = np.full((P, tot_chunks), 255, np.uint8)
        idx16 = np.zeros((16, tot_s), np.int16)
        counts = np.zeros(ncalls, np.int32)
        for b in range(BPC):
            s, lo_idx, hi_idx = lists[c][b]
            co = chunk_off[b]
            for half, sub, m_half, half_chunk_base, relab in (
                (0, lo_idx, m_lo[b], 0, lo_of),
                (1, hi_idx, m_hi[b], m_lo[b], hi_of),
            ):
                g_sz = int(m_half) * P
                # trailing -1 padding is skipped by the ucode via
                # num_idxs_reg (KCNT=1 only; measured net-negative: the
                # ucode's register read costs ~1us/call).  Default pads
                # with row 0 and full immediate counts.
                nreal = len(sub)
                rows_h = np.full(g_sz, -1 if PAD_SKIP else 0, np.int64)
                rows_h[: nreal] = relab[s + sub]
                cis = ci_lo[b] if half == 0 else ci_hi[b]
                for j, cc in enumerate(cis):
                    w = min(GMAX, int(m_half) - j * GMAX) * P
                    cnt = max(0, min(w, nreal - j * GMAX * P))
                    if cnt == 0:
                        if PAD_SKIP:
                            rows_h[j * GMAX * P] = 0
                        cnt = 1
                    counts[cc] = cnt
                cols_h = np.full(g_sz, 255, np.int64)    # pad col -> 255
                cols_h[: len(sub)] = col_s[s + sub] - (c * BPC + b) * P
                ea_h = np.zeros((g_sz, DE), np.float32)
                ea_h[: len(sub)] = ea_sorted[s + sub]
                ii = np.arange(g_sz)
                pp, jj = ii % P, ii // P
                eaT[:, co + half_chunk_base + jj, pp] = ea_h.T
                colrel[pp, co + half_chunk_base + jj] = cols_h
                # 16-wrapped idx at column offset
                so = s_off[b] + (0 if half == 0 else s_lo[b])
                idx16[ii % 16, so + ii // 16] = rows_h
        per_core.append(
            dict(
                eaT=_bf16(eaT),
                colrel=colrel,
                idx16=np.tile(idx16, (8, 1)),            # replicate to 128 partitions
                gcounts=counts.reshape(1, ncalls),
            )
        )

    sched = dict(
        m_lo=[int(v) for v in m_lo], m_hi=[int(v) for v in m_hi],
        chunk_off=[int(v) for v in chunk_off], s_off=[int(v) for v in s_off],
        s_lo=[int(v) for v in s_lo],
        tot_chunks=tot_chunks, tot_s=tot_s,
        ci_lo=ci_lo, ci_hi=ci_hi, ncalls=ncalls, gmax=GMAX,
    )
    return per_core, sched, dinv


# ---------------- device program ----------------

def _build(sched):
    import os
    NQ = int(os.environ.get("KQ", "1"))
    nc = bacc.Bacc(None, target_bir_lowering=False, debug=False,
                   num_swdge_queues=NQ)
    TC, TS = sched["tot_chunks"], sched["tot_s"]
    GMAX = sched["gmax"]                       # chunks per dma_gather call
    NCALLS = sched["ncalls"]
    ci_lo, ci_hi = sched["ci_lo"], sched["ci_hi"]
    SINGLE_PKT = os.environ.get("KSINGLEPKT", "0") == "1"
    USE_CNT = os.environ.get("KCNT", "0") == "1"
    SKIP_GATHER = os.environ.get("KSKIP_GATHER") == "1"
    SKIP_CC = os.environ.get("KSKIP_CC") == "1"
    SKIP_EDGE = os.environ.get("KSKIP_EDGE") == "1"
    SKIP_EA = os.environ.get("KSKIP_EA") == "1"

    # ---- external inputs (per-core shapes) ----
    decl = nc.declare_dram_parameter
    xT = decl("xT", [P, NSH], dt.bfloat16, isOutput=False)
    eaT_d = decl("eaT", [DE, TC, P], dt.bfloat16, isOutput=False)
    colrel_d = decl("colrel", [P, TC], dt.uint8, isOutput=False)
    idx_d = decl("idx16", [P, TS], dt.int16, isOutput=False)
    dinv_d = decl("dinvt", [P, NSH], dt.bfloat16, isOutput=False)
    dinvnm_d = decl("dinv_nm", [P, BPC], dt.float32, isOutput=False)
    iota_d = decl("iota_u8", [P, P], dt.uint8, isOutput=False)
    ident_d = decl("ident", [P, P], dt.bfloat16, isOutput=False)
    W_d = [decl(f"W{k}", [D, D], dt.bfloat16, isOutput=False) for k in (1, 2, 3)]
    Wf_d = [decl(f"Wf{k}", [D, D], dt.float32, isOutput=False) for k in (2, 3)]
    Wl_d = decl("Wlin", [D, D], dt.bfloat16, isOutput=False)
    Wlf_d = decl("Wlinf", [D, D], dt.float32, isOutput=False)
    We_d = [decl(f"We{k}", [DE, D], dt.bfloat16, isOutput=False) for k in (1, 2, 3)]
    # b_tot[k] = b_k + be_k as row [1,128]; bf16 row for layer 1 direct use
    brow1bf_d = decl("brow1bf", [1, D], dt.bfloat16, isOutput=False)
    brow_d = [decl(f"brow{k}", [1, D], dt.float32, isOutput=False) for k in (2, 3)]
    blrow_d = decl("blrow", [1, D], dt.float32, isOutput=False)
    g_d = [decl(f"g{k}", [D, 1], dt.float32, isOutput=False) for k in (1, 2, 3)]
    bt_d = [decl(f"bt{k}", [D, 1], dt.float32, isOutput=False) for k in (1, 2, 3)]
    gcnt_d = decl("gcounts", [1, NCALLS], dt.int32, isOutput=False)
    out_d = decl("out_nm", [NSH, D], dt.float32, isOutput=True)

    rg = [list(range(NCORES))]

    with tile.TileContext(nc) as tc:
        import contextlib
        with contextlib.ExitStack() as ctx:
            ek = ctx.enter_context
            const = ek(tc.tile_pool(name="const", bufs=1))
            nodeb = ek(tc.tile_pool(name="nodeb", bufs=3))
            edge_ea = ek(tc.tile_pool(name="edge_ea", bufs=4))
            edge_idx = ek(tc.tile_pool(name="edge_idx", bufs=4))
            edge_hr = ek(tc.tile_pool(name="edge_hr", bufs=3))
            edge_msg = ek(tc.tile_pool(name="edge_msg", bufs=2))
            edge_S = ek(tc.tile_pool(name="edge_S", bufs=2))
            small = ek(tc.tile_pool(name="small", bufs=4))
            trp = ek(tc.tile_pool(name="trp", bufs=3))
            ps_mp = ek(tc.tile_pool(name="ps_mp", bufs=3, space="PSUM"))
            ps_conv = ek(tc.tile_pool(name="ps_conv", bufs=2, space="PSUM"))
            ps_node = ek(tc.tile_pool(name="ps_node", bufs=2, space="PSUM"))
            ps_row = ek(tc.tile_pool(name="ps_row", bufs=1, space="PSUM"))
            dram = ek(tc.tile_pool(name="dram", bufs=2, space="DRAM"))

            # ---- load constants ----
            def ld(pool, shape, dty, src, name):
                t = pool.tile(shape, dty, name=name)
                nc.sync.dma_start(out=t[:], in_=src[...])
                return t

            xT_t = ld(const, [P, NSH], dt.bfloat16, xT, 'xT_t')
            W_t = [ld(const, [D, D], dt.bfloat16, W_d[i], f'W_t{i}') for i in range(3)]
            brow1bf_t = ld(const, [1, D], dt.bfloat16, brow1bf_d, 'brow1bf_t')
            dinvnm_t = ld(const, [P, BPC], dt.float32, dinvnm_d, 'dinvnm_t')
            dinv_t = ld(const, [P, NSH], dt.bfloat16, dinv_d, 'dinv_d_t')
            iota_t = ld(const, [P, P], dt.uint8, iota_d, 'iota_d_t')
            ident_t = ld(const, [P, P], dt.bfloat16, ident_d, 'ident_d_t')
            colrel_t = ld(const, [P, TC], dt.uint8, colrel_d, 'colrel_d_t')
            Wf_t = [ld(const, [D, D], dt.float32, Wf_d[i], f'Wf_t{i}') for i in range(2)]
            Wl_t = ld(const, [D, D], dt.bfloat16, Wl_d, 'Wl_d_t')
            Wlf_t = ld(const, [D, D], dt.float32, Wlf_d, 'Wlf_d_t')
            We_t = [ld(const, [DE, D], dt.bfloat16, We_d[i], f'We_t{i}') for i in range(3)]
            brow_t = [ld(const, [1, D], dt.float32, brow_d[i], f'brow_t{i}') for i in range(2)]
            blrow_t = ld(const, [1, D], dt.float32, blrow_d, 'blrow_d_t')
            g_t = [ld(const, [D, 1], dt.float32, g_d[i], f'g_t{i}') for i in range(3)]
            bt_t = [ld(const, [D, 1], dt.float32, bt_d[i], f'bt_t{i}') for i in range(3)]

            t_T = [const.tile([P, NSH], dt.bfloat16, name=f't_T{i}') for i in range(2)]
            eps_t = const.tile([P, 1], dt.float32, name='eps_t')
            nc.vector.memset(eps_t[:], EPS)
            ones_t = const.tile([1, P], dt.bfloat16, name='ones_t')
            nc.vector.memset(ones_t[:], 1.0)
            gcnt_t = ld(const, [1, NCALLS], dt.int32, gcnt_d, 'gcnt_t')
            cnt_reg = nc.alloc_register(mybir.EngineType.Pool, "gcnt_reg")

            def gather_count(cc, default):
                if not USE_CNT:
                    return default
                nc.gpsimd.reg_load(cnt_reg, gcnt_t[0:1, cc:cc + 1])
                return cnt_reg

            m_lo, m_hi = sched["m_lo"], sched["m_hi"]
            chunk_off, s_off, s_lo = sched["chunk_off"], sched["s_off"], sched["s_lo"]

            def node_linear(rhs_w, brow_bf, src_t, shard, final=False,
                            after_block=None):
                """Per block: psum = src_blk.T @ W + ones x brow; scale by
                dinv (not for final); write node-major to shard/out."""
                for b in range(BPC):
                    pp = ps_node.tile([P, P], dt.float32, space="PSUM",
                                      padded_shape=[P, 512])
                    nc.tensor.matmul(out=pp[:], lhsT=src_t[:, b * P:(b + 1) * P],
                                     rhs=rhs_w[:], start=True, stop=False,
                                     skip_group_check=True)
                    nc.tensor.matmul(out=pp[:], lhsT=ones_t[:], rhs=brow_bf[:],
                                     start=False, stop=True, skip_group_check=True)
                    if final:
                        ot = nodeb.tile([P, P], dt.float32)
                        nc.scalar.activation(out=ot[:], in_=pp[:], func=AF.Identity)
                    else:
                        ot = nodeb.tile([P, P], dt.bfloat16)
                        nc.scalar.activation(out=ot[:], in_=pp[:], func=AF.Identity,
                                             scale=dinvnm_t[:, b:b + 1])
                    nc.sync.dma_start(out=shard[b * P:(b + 1) * P, :], in_=ot[:])
                    if after_block and b in after_block:
                        after_block[b]()

            for k in range(3):  # layers 1..3
                # ---- fold previous BN (k>=1) into this layer's weights ----
                if k == 0:
                    Wp_t, brow_bf = W_t[0], brow1bf_t
                    src_t = xT_t
                else:
                    a_t, c_t = bn_fold  # from previous layer epilogue
                    Wp_t = small.tile([D, D], dt.bfloat16)
                    nc.scalar.activation(out=Wp_t[:], in_=W_t[k][:], func=AF.Identity,
                                         scale=a_t[:])
                    pb = ps_row.tile([1, D], dt.float32, space="PSUM",
                                     padded_shape=[1, 512])
                    nc.tensor.matmul(out=pb[:], lhsT=c_t[:], rhs=Wf_t[k - 1][:],
                                     start=True, stop=True)
                    browf = small.tile([1, D], dt.float32)
                    nc.vector.tensor_tensor(out=browf[:], in0=pb[:],
                                            in1=brow_t[k - 1][:], op=ALU.add)
                    brow_bf = small.tile([1, D], dt.bfloat16)
                    nc.scalar.activation(out=brow_bf[:], in_=browf[:], func=AF.Identity)
                    src_t = t_T[(k - 1) % 2]

                # ---- node linear (node-major shard) + split AllGather ----
                shard = dram.tile([NSH, D], dt.bfloat16)
                table_lo = dram.tile([TLO, D], dt.bfloat16, addr_space="Shared")
                table_hi = dram.tile([THI, D], dt.bfloat16, addr_space="Shared")

                def ag(piece, tab):
                    if not SKIP_CC:
                        nc.gpsimd.collective_compute(
                            "AllGather", ALU.bypass, replica_groups=rg,
                            ins=[piece.opt()], outs=[tab[:].opt()],
                        )
                    else:
                        nc.sync.dma_start(out=tab[:piece.shape[0], :], in_=piece)

                node_linear(Wp_t, brow_bf, src_t, shard, after_block={
                    LOB - 1: lambda: ag(shard[:LOROWS, :], table_lo),
                    BPC - 1: lambda: ag(shard[LOROWS:, :], table_hi),
                })

                # ---- edge phase over 49 dest blocks (software-pipelined:
                # lo gathers run DEPTH blocks ahead so the hi-table AllGather
                # and per-block compute hide under the gather stream) ----
                sums_t = small.tile([P, BPC], dt.float32)
                sqs_t = small.tile([P, BPC], dt.float32)
                tnew = t_T[k % 2]
                DEPTH = 2
                inflight = {}

                def stage_load_lo(b):
                    m = m_lo[b] + m_hi[b]
                    co = chunk_off[b]
                    ea_t = edge_ea.tile([DE, m, P], dt.bfloat16)
                    if not SKIP_EA:
                        nc.sync.dma_start(out=ea_t[:], in_=eaT_d[:, co:co + m, :])
                    else:
                        nc.vector.memset(ea_t[:, :1, :], 0.0)
                    stot_b = s_lo[b] + m_hi[b] * 8
                    idx_t = edge_idx.tile([P, stot_b], dt.int16, name=f'idxb')
                    nc.sync.dma_start(out=idx_t[:], in_=idx_d[:, s_off[b]:s_off[b] + stot_b])
                    hr_t = edge_hr.tile([P, m, D], dt.bfloat16)
                    if USE_CNT:
                        # skipped padding slots must hold finite values for
                        # the identity-add matmul (0 x one-hot stays 0)
                        nc.vector.memset(hr_t[:], 0.0)
                    if not SKIP_GATHER:
                        for j, pc in enumerate(range(0, m_lo[b], GMAX)):
                            pw = min(GMAX, m_lo[b] - pc)
                            nc.gpsimd.dma_gather(
                                out_ap=hr_t[:, pc:pc + pw, :],
                                in_ap=table_lo[:, :],
                                idxs_ap=idx_t[:, pc * 8:(pc + pw) * 8],
                                num_idxs=pw * P,
                                num_idxs_reg=gather_count(ci_lo[b][j], pw * P),
                                elem_size=D,
                                single_packet=SINGLE_PKT,
                                queue_num=ci_lo[b][j] % NQ,
                            )
                    else:
                        nc.vector.memset(hr_t[:, :1, :], 0.25)
                    inflight[b] = (ea_t, idx_t, hr_t)

                def stage_compute(b):
                    m = m_lo[b] + m_hi[b]
                    co = chunk_off[b]
                    ea_t, idx_t, hr_t = inflight.pop(b)
                    if not SKIP_GATHER:
                        for j, pc in enumerate(range(0, m_hi[b], GMAX)):
                            pw = min(GMAX, m_hi[b] - pc)
                            nc.gpsimd.dma_gather(
                                out_ap=hr_t[:, m_lo[b] + pc:m_lo[b] + pc + pw, :],
                                in_ap=table_hi[:, :],
                                idxs_ap=idx_t[:, s_lo[b] + pc * 8:s_lo[b] + (pc + pw) * 8],
                                num_idxs=pw * P,
                                num_idxs_reg=gather_count(ci_hi[b][j], pw * P),
                                elem_size=D,
                                single_packet=SINGLE_PKT,
                                queue_num=ci_hi[b][j] % NQ,
                            )
                    # S indicator [P, m, P]
                    S_t = edge_S.tile([P, m, P], S_DTYPE)
                    iota_b = bass.AP(tensor=iota_t.tensor, offset=iota_t[:].offset,
                                     ap=[iota_t[:].ap[0], [0, m], iota_t[:].ap[1]])
                    cr = colrel_t[:, co:co + m]
                    cr_b = bass.AP(tensor=colrel_t.tensor, offset=cr.offset,
                                   ap=[cr.ap[0], cr.ap[1], [0, P]])
                    nc.vector.tensor_tensor(out=S_t[:], in0=iota_b, in1=cr_b,
                                            op=ALU.is_equal)
                    # messages
                    msg_t = edge_msg.tile([P, m, D], dt.bfloat16)
                    if SKIP_EDGE:
                        nc.vector.memset(msg_t[:, :1, :], 0.1)
                    j = 0 if not SKIP_EDGE else m
                    while j < m:
                        jw = min(4, m - j)
                        mp = ps_mp.tile([P, 4, D], dt.float32, space="PSUM")
                        # start=True zeroes the whole 2KB bank, so the
                        # full-tile identity-add must come first.
                        nc.tensor.matmul(
                            out=mp[:, :jw, :].rearrange("p j d -> p (j d)"),
                            lhsT=ident_t[:],
                            rhs=hr_t[:, j:j + jw, :].rearrange("p j d -> p (j d)"),
                            start=True, stop=False, skip_group_check=True)
                        for jj in range(jw):
                            nc.tensor.matmul(out=mp[:, jj, :],
                                             lhsT=ea_t[:, j + jj, :], rhs=We_t[k][:],
                                             start=False, stop=(jj == jw - 1),
                                             skip_group_check=True)
                        nc.scalar.activation(
                            out=msg_t[:, j:j + jw, :].rearrange("p j d -> p (j d)"),
                            in_=mp[:, :jw, :].rearrange("p j d -> p (j d)"), func=AF.Relu)
                        j += jw
                    # scatter into conv accumulator (feature-major out)
                    cp = ps_conv.tile([P, P], dt.float32, space="PSUM",
                                      padded_shape=[P, 512])
                    for j in range(m if not SKIP_EDGE else 1):
                        nc.tensor.matmul(out=cp[:], lhsT=msg_t[:, j, :],
                                         rhs=S_t[:, j, :],
                                         start=(j == 0), stop=(j == m - 1))
                    # epilogue: dinv scale, relu -> t, stats
                    sl = slice(b * P, (b + 1) * P)
                    pre = trp.tile([P, P], dt.float32)
                    nc.vector.tensor_tensor(out=pre[:], in0=cp[:],
                                            in1=dinv_t[:, sl], op=ALU.mult)
                    nc.scalar.activation(out=tnew[:, sl], in_=pre[:], func=AF.Relu,
                                         accum_out=sums_t[:, b:b + 1])
                    sq_scr = trp.tile([P, P], dt.bfloat16)
                    nc.scalar.activation(out=sq_scr[:], in_=tnew[:, sl], func=AF.Square,
                                         accum_out=sqs_t[:, b:b + 1])

                for b in range(BPC + DEPTH):
                    if b < BPC:
                        stage_load_lo(b)
                    if b >= DEPTH:
                        stage_compute(b - DEPTH)

                # ---- BN stats + fold coefficients ----
                st = small.tile([P, 2], dt.float32)
                nc.vector.tensor_reduce(out=st[:, 0:1], in_=sums_t[:],
                                        axis=mybir.AxisListType.X, op=ALU.add)
                nc.vector.tensor_reduce(out=st[:, 1:2], in_=sqs_t[:],
                                        axis=mybir.AxisListType.X, op=ALU.add)
                st_in = dram.tile([P, 2], dt.float32)
                st_out = dram.tile([P, 2], dt.float32)
                nc.sync.dma_start(out=st_in[:], in_=st[:])
                if not SKIP_CC:
                    nc.gpsimd.collective_compute(
                        "AllReduce", ALU.add, replica_groups=rg,
                        ins=[st_in[:].opt()], outs=[st_out[:].opt()],
                    )
                else:
                    nc.sync.dma_start(out=st_out[:, :], in_=st_in[:, :])
                stg = small.tile([P, 2], dt.float32)
                nc.sync.dma_start(out=stg[:], in_=st_out[:])
                mu = small.tile([P, 1], dt.float32)
                nc.vector.tensor_scalar(out=mu[:], in0=stg[:, 0:1], scalar1=1.0 / N,
                                        scalar2=None, op0=ALU.mult)
                ex2 = small.tile([P, 1], dt.float32)
                nc.vector.tensor_scalar(out=ex2[:], in0=stg[:, 1:2], scalar1=1.0 / N,
                                        scalar2=None, op0=ALU.mult)
                var = small.tile([P, 1], dt.float32)
                nc.vector.tensor_tensor(out=var[:], in0=mu[:], in1=mu[:], op=ALU.mult)
                nc.vector.tensor_tensor(out=var[:], in0=ex2[:], in1=var[:],
                                        op=ALU.subtract)
                sd = small.tile([P, 1], dt.float32)
                nc.scalar.activation(out=sd[:], in_=var[:], func=AF.Sqrt, bias=eps_t[:])
                rs = small.tile([P, 1], dt.float32)
                nc.vector.reciprocal(out=rs[:], in_=sd[:])
                a_t = small.tile([P, 1], dt.float32)
                nc.vector.tensor_tensor(out=a_t[:], in0=rs[:], in1=g_t[k][:],
                                        op=ALU.mult)
                c_t = small.tile([P, 1], dt.float32)
                nc.vector.tensor_tensor(out=c_t[:], in0=mu[:], in1=a_t[:], op=ALU.mult)
                nc.vector.tensor_tensor(out=c_t[:], in0=bt_t[k][:], in1=c_t[:],
                                        op=ALU.subtract)
                bn_fold = (a_t, c_t)

            # ---- final linear: out = t3_blk.T @ Wl' + ones x bl' (node-major) ----
            a_t, c_t = bn_fold
            Wlp = small.tile([D, D], dt.bfloat16)
            nc.scalar.activation(out=Wlp[:], in_=Wl_t[:], func=AF.Identity, scale=a_t[:])
            pb = ps_row.tile([1, D], dt.float32, space="PSUM", padded_shape=[1, 512])
            nc.tensor.matmul(out=pb[:], lhsT=c_t[:], rhs=Wlf_t[:], start=True, stop=True)
            blrowf = small.tile([1, D], dt.float32)
            nc.vector.tensor_tensor(out=blrowf[:], in0=pb[:], in1=blrow_t[:], op=ALU.add)
            blrow_bf = small.tile([1, D], dt.bfloat16)
            nc.scalar.activation(out=blrow_bf[:], in_=blrowf[:], func=AF.Identity)
            node_linear(Wlp, blrow_bf, t_T[0], out_d, final=True)

    nc.finalize()
    return nc


# ---------------- public entry point ----------------

_CACHE = {}
LAST_EXEC_NS = None


def _make_in_maps(inputs, per_core, dinv):
    x = np.asarray(inputs["x"], np.float32)

    dinv_pad = np.zeros(NP, np.float32)
    dinv_pad[:N] = dinv
    xT_full = np.zeros((P, NP), np.float32)
    xT_full[:, :N] = x.T

    Ws = {k: np.asarray(inputs[k], np.float32) for k in
          ("W1", "W2", "W3", "Wl", "We1", "We2", "We3")}
    bt_tot = {k: np.asarray(inputs[f"b{k}"], np.float32) +
                 np.asarray(inputs[f"be{k}"], np.float32) for k in (1, 2, 3)}

    in_maps = []
    for c in range(NCORES):
        sl = slice(c * NSH, (c + 1) * NSH)
        im = dict(per_core[c])
        im["xT"] = _bf16(xT_full[:, sl])
        im["dinvt"] = _bf16(np.tile(dinv_pad[sl][None, :], (P, 1)))
        im["dinv_nm"] = np.ascontiguousarray(
            dinv_pad[sl].reshape(BPC, P).T.astype(np.float32))
        im["iota_u8"] = np.tile(np.arange(P, dtype=np.uint8)[None, :], (P, 1))
        im["ident"] = _bf16(np.eye(P))
        for i, k in enumerate((1, 2, 3)):
            im[f"W{k}"] = _bf16(Ws[f"W{k}"])
            im[f"We{k}"] = _bf16(Ws[f"We{k}"])
            im[f"g{k}"] = np.asarray(inputs[f"g{k}"], np.float32).reshape(D, 1)
            im[f"bt{k}"] = np.asarray(inputs[f"bt{k}"], np.float32).reshape(D, 1)
        im["Wf2"] = Ws["W2"]
        im["Wf3"] = Ws["W3"]
        im["Wlin"] = _bf16(Ws["Wl"])
        im["Wlinf"] = Ws["Wl"]
        im["brow1bf"] = _bf16(bt_tot[1].reshape(1, D))
        im["brow2"] = bt_tot[2].reshape(1, D)
        im["brow3"] = bt_tot[3].reshape(1, D)
        im["blrow"] = np.asarray(inputs["bl"], np.float32).reshape(1, D)
        in_maps.append(im)
    return in_maps


def kernel(**inputs):
    edge_attr = np.asarray(inputs["edge_attr"], np.float32)
    edge_index = np.asarray(inputs["edge_index"])

    per_core, sched, dinv = _preprocess(edge_index, edge_attr)
    in_maps = _make_in_maps(inputs, per_core, dinv)

    import os
    key = ("k", os.environ.get("KCNT", "0"), os.environ.get("KQ", "1"), os.environ.get("KSINGLEPKT", "0"), sched["tot_chunks"], sched["tot_s"],
           tuple(sched["m_lo"]), tuple(sched["m_hi"]))
    if key not in _CACHE:
        _CACHE[key] = _build(sched)
    nc = _CACHE[key]

    trace = os.environ.get("KPROF") == "1"
    tdir = os.environ.get("KTRACEDIR") or None
    r = run_bass_kernel_spmd(nc, in_maps, core_ids=list(range(NCORES)), trace=trace,
                             tmpdir=tdir)
    if trace:
        print(f"HW exec time: {r.exec_time_ns} ns", flush=True)
        global LAST_EXEC_NS
        LAST_EXEC_NS = r.exec_time_ns
    res = r.results
    out = np.concatenate([res[c]["out_nm"] for c in range(NCORES)], axis=0)  # [NP, D]
    return np.ascontiguousarray(out[:N]).astype(np.float32)
